# revision 47
# baseline (speedup 1.0000x reference)
"""GAT layer (PyG GATConv semantics) on 8 Trainium2 NeuronCores.

Strategy (edge/graph parallel, dst-sharded, v3):
  - Append self-loops; partition destination nodes into 784 windows of 128.
  - Rank windows by edge count; window rank-group g supplies slot g of each
    of the 8 cores, so all cores share one compile-time schedule.
  - Each core builds the full node table T1c[chunk] (4 chunks of 25088 rows,
    512B rows = [h(128 bf16, c-major heads) | a_src(4) | junk]) via
    x @ [W | W@A] (bf16) on the TensorEngine.  Rows are partition-major
    (row = (n%128)*196 + n//128) so table writes are 128 contiguous 3.5KB
    descriptors per block.
  - A per-window a_dst table a2[d,4] lives in SBUF (from the xsT input).
  - Edge phase (chunk-major, software-pipelined emission so the in-order
    engine queues overlap; also overlaps the remaining table builds): one
    dma_gather of T1 rows per (superblock, chunk) segment.  Per (slot,
    chunk) cell, full 128-edge tiles are exclusive; the partial remainders
    of a superblock's cells share gather tiles, with each cell keeping its
    own one-hot "view" (others' lanes masked to dstloc -1).
  - Per-edge a_dst on-chip: Sel[e,(d,v)] = (dl[e,v] == d) built d-major on
    DVE (2-byte packed operands = 2x path), SelT via PE transposes (psum
    evicted by the scalar engine), adps[tile] += SelT_v.T @ a2[slot_v]
    (same-region uninterrupted psum chains only - interleaved chains and
    in-place DVE normalize corrupted results on real HW).
    w = exp(max(z, 0.2z)); messages h*w (w broadcast on a middle dim) are
    written in place into the gather buffer; psum[d] += Sel_v.T @ [h*w | w]
    per view, accumulated into an SBUF f32 accumulator per slot.
  - Output (normalize + bias) is emitted incrementally per superblock as
    its last chunk finishes.
"""

import math
import os

import numpy as np
import ml_dtypes

import concourse.bacc as bacc
import concourse.bass as bass
import concourse.mybir as mybir
import concourse.tile as tile
from concourse.library_config import mlp
from concourse.bass_utils import run_bass_kernel_spmd
from concourse.masks import make_identity

BF16 = ml_dtypes.bfloat16

N = 100000
E = 1600000
IN_DIM = 128
HEADS = 4
CDIM = 32
NCORES = 8
P = 128

NP_ = 100352            # N padded to 784 x-tiles of 128
NWIN = NP_ // P         # 784 global windows
SLOTS = NWIN // NCORES  # 98 slots per core
CHUNK = NP_ // 4        # 25088 rows per T1 chunk (int16-indexable)
TPC = CHUNK // P        # 196 tiles (rows-per-partition) per chunk
SHARD = SLOTS * P       # 12544 dst nodes per core
SUPB = 7                # slots per superblock (gather batching)
NBLK = 7                # table-build tiles per block (196 = 28*7)
NCHUNK = 4


# ---------------------------------------------------------------------------
# walrus workaround: this container's walrus accepts ONE sem wait per
# instruction; TileContext's tail drain accumulates many. Split extras onto
# single-wait EventSemaphore instructions.
def _split_multi_waits(nc):
    n = [0]

    def fresh():
        n[0] += 1
        return f"waitsplit-{n[0]}"

    for fn in nc.m.functions:
        for bb in fn.blocks:
            insts = list(bb.instructions)
            if not any(
                i.sync_info is not None and len(i.sync_info.on_wait) > 1
                for i in insts
            ):
                continue
            out = []
            for inst in insts:
                si = inst.sync_info
                if si is not None and len(si.on_wait) > 1:
                    waits = list(si.on_wait)
                    for w in waits[:-1]:
                        out.append(mybir.InstEventSemaphore(
                            name=fresh(), opcode="EventSemaphore",
                            engine=inst.engine,
                            sync_info=mybir.SyncInfo(on_wait=[w], on_update=[]),
                        ))
                    si.on_wait = waits[-1:]
                out.append(inst)
            bb.instructions = out


def _wrap_idx(seg):
    """dma_gather index layout: wrap in 16 partitions, replicate x8."""
    assert seg.size % 128 == 0
    return np.tile(seg.reshape(-1, 16).T, (8, 1)).astype(np.int16)


# ---------------------------------------------------------------------------
def _host_prep(x, edge_index):
    """Build the per-core schedule + data arrays. Pure indexing, no FP math."""
    src = np.concatenate([edge_index[0].astype(np.int64), np.arange(N)])
    dst = np.concatenate([edge_index[1].astype(np.int64), np.arange(N)])
    win = dst >> 7

    wcount = np.bincount(win, minlength=NWIN)
    order = np.argsort(-wcount, kind="stable")        # windows by size desc
    core_of_win = np.empty(NWIN, np.int64)
    slot_of_win = np.empty(NWIN, np.int64)
    core_of_win[order] = np.arange(NWIN) % NCORES
    slot_of_win[order] = np.arange(NWIN) // NCORES

    chunk = src // CHUNK
    wc = np.bincount(win * 4 + chunk, minlength=NWIN * 4).reshape(NWIN, 4)
    # caps[g][c]: tiles for chunk-c segment of slot g (max over the 8 cores)
    grp = order.reshape(SLOTS, NCORES)
    caps = np.ceil(wc[grp].max(axis=1) / P).astype(np.int64)   # [SLOTS, 4]

    mx = wc[grp].max(axis=1)                          # [SLOTS, 4] edges (max/core)

    # stream layout, chunk-major; remainders of a superblock's cells share
    # gather tiles (each cell keeps its own one-hot view of shared tiles).
    supb_sizes = [SUPB] * (SLOTS // SUPB) + ([SLOTS % SUPB] if SLOTS % SUPB else [])
    segs = []               # per (c, sb): dict
    gstart = np.zeros((SLOTS, 4), np.int64)   # first exclusive gather tile
    vstart = np.zeros((SLOTS, 4), np.int64)   # first exclusive view col
    gshared = np.zeros((SLOTS, 4), np.int64)  # shared gather tile
    vshared = np.zeros((SLOTS, 4), np.int64)  # shared view col
    lanelo = np.zeros((SLOTS, 4), np.int64)   # lane offset in shared tile
    gcur = 0
    vcur = 0
    for c in range(4):
        sb0 = 0
        for sb, nsl in enumerate(supb_sizes):
            cells = [g for g in range(sb0, sb0 + nsl) if mx[g, c] > 0]
            g0, v0 = gcur, vcur
            # exclusive full tiles + view cols
            for g in cells:
                F = int(mx[g, c]) // P
                gstart[g, c] = gcur
                vstart[g, c] = vcur
                gcur += F
                vcur += F
            # shared tiles: first-fit of remainders
            stiles = []      # list of filled lanes per shared tile
            for g in cells:
                r = int(mx[g, c]) % P
                if r == 0:
                    continue
                ti = next((i for i, f in enumerate(stiles) if f + r <= P),
                          None)
                if ti is None:
                    ti = len(stiles)
                    stiles.append(0)
                gshared[g, c] = gcur + ti
                lanelo[g, c] = stiles[ti]
                vshared[g, c] = vcur
                stiles[ti] += r
                vcur += 1
            gcur += len(stiles)
            Lg, Lv = gcur - g0, vcur - v0
            vtile = np.zeros(Lv, np.int64)
            runs = []
            for g in cells:
                F = int(mx[g, c]) // P
                vis = list(range(vstart[g, c] - v0, vstart[g, c] - v0 + F))
                vtile[vstart[g, c] - v0:vstart[g, c] - v0 + F] = (
                    np.arange(gstart[g, c] - g0, gstart[g, c] - g0 + F))
                if int(mx[g, c]) % P:
                    vis.append(vshared[g, c] - v0)
                    vtile[vshared[g, c] - v0] = gshared[g, c] - g0
                runs.append((g, vis))
            if Lg:
                segs.append({"c": c, "sb": sb, "Lg": Lg, "Lv": Lv,
                             "g0": g0, "v0": v0, "vtile": vtile.tolist(),
                             "runs": runs})
            sb0 += nsl
    T_g, T_v = gcur, vcur

    # per-core arrays
    ecore = core_of_win[win]
    eslot = slot_of_win[win]
    cores = []
    for k in range(NCORES):
        m = np.nonzero(ecore == k)[0]
        es, ed, ec, eg = src[m], dst[m], chunk[m], eslot[m]
        o = np.lexsort((ed, ec, eg))
        es, ed, ec, eg = es[o], ed[o], ec[o], eg[o]
        # rank within (slot, chunk) group
        key = eg * 4 + ec
        start = np.searchsorted(key, np.arange(SLOTS * 4))
        rank = np.arange(len(es)) - start[key]
        F = mx[eg, ec] // P
        excl = rank < F * P
        gt = np.where(excl, gstart[eg, ec] + (rank >> 7),
                      gshared[eg, ec])
        lane = np.where(excl, rank & 127, lanelo[eg, ec] + rank - F * P)
        vi = np.where(excl, vstart[eg, ec] + (rank >> 7), vshared[eg, ec])
        esl = es - ec * CHUNK                         # chunk-local node id
        row = (esl & 127) * TPC + (esl >> 7)          # partition-major T1 row
        g1 = np.zeros(T_g * P, np.int16)              # pad: row 0 (masked)
        dl = np.full(T_v * P, -1, np.int8)            # pad: dstloc -1
        g1[gt * P + lane] = row.astype(np.int16)
        dl[vi * P + lane] = (ed & 127).astype(np.int8)
        cores.append({"g1": g1, "dl": dl})

    sched = {
        "mx": mx, "supb_sizes": supb_sizes, "segs": segs,
        "T_g": T_g, "T_v": T_v, "order": order, "grp": grp,
        "core_of_win": core_of_win, "slot_of_win": slot_of_win,
    }
    return cores, sched


def _pack_core_arrays(core, sched):
    """Wrap index streams per gather instruction; dstloc per view column."""
    g1_parts = []
    for sg in sched["segs"]:
        t0, tiles = sg["g0"], sg["Lg"]
        g1_parts.append(_wrap_idx(core["g1"][t0 * P:(t0 + tiles) * P]))
    g1w = np.concatenate(g1_parts, axis=1) if g1_parts else np.zeros((128, 0), np.int16)
    dlt = core["dl"].reshape(sched["T_v"], P).T.astype(BF16)   # [128, T_v]
    return g1w, dlt


# ---------------------------------------------------------------------------
def _build_nc(sched):
    mx = sched["mx"]
    supb_sizes = sched["supb_sizes"]
    T_g, T_v = sched["T_g"], sched["T_v"]
    AF = mybir.ActivationFunctionType
    AL = mybir.AluOpType
    f32, bf16 = mybir.dt.float32, mybir.dt.bfloat16
    i16, i8 = mybir.dt.int16, mybir.dt.int8

    g1cols = sum(sg["Lg"] * 8 for sg in sched["segs"])
    LCMAX = max(sg["Lv"] for sg in sched["segs"])
    # first chunk with edges, per slot (acc copy-vs-add selector)
    first_c = [int(np.nonzero(mx[g])[0][0]) for g in range(SLOTS)]

    nc = bacc.Bacc("TRN2")
    xT = nc.declare_dram_parameter("xT", [P, NP_], bf16, isOutput=False)
    xsT = nc.declare_dram_parameter("xsT", [P, SHARD], bf16, isOutput=False)
    Wp = nc.declare_dram_parameter("W", [P, P], bf16, isOutput=False)
    Acat = nc.declare_dram_parameter("Acat", [P, 8], bf16, isOutput=False)
    biasr = nc.declare_dram_parameter("biasr", [P, P], f32, isOutput=False)
    iotbp = nc.declare_dram_parameter("iotB", [P, P * LCMAX], bf16, isOutput=False)
    g1i = nc.declare_dram_parameter("g1i", [P, max(g1cols, 8)], i16, isOutput=False)
    dlp = nc.declare_dram_parameter("dlp", [P, max(T_v, 1)], bf16, isOutput=False)
    outp = nc.declare_dram_parameter("out", [SHARD, P], f32, isOutput=True)

    T1c = [nc.dram_tensor(f"t1c{c}", [CHUNK, 256], bf16) for c in range(NCHUNK)]

    nc.gpsimd.load_library(mlp)

    _PH = int(os.environ.get("GAT_PHASES", "3"))

    with tile.TileContext(nc) as tc:
        with tc.tile_pool(name="const", bufs=1) as cpool:
            ident = cpool.tile([P, P], f32)
            make_identity(nc, ident[:])
            identb = cpool.tile([P, P], bf16)
            make_identity(nc, identb[:])
            iotB = cpool.tile([P, P * LCMAX], bf16)
            nc.sync.dma_start(out=iotB[:], in_=iotbp[:])
            bias_t = cpool.tile([P, P], f32)
            nc.sync.dma_start(out=bias_t[:], in_=biasr[:])
            wprime = cpool.tile([P, 136], bf16)
            nc.sync.dma_start(out=wprime[:, 0:128], in_=Wp[:])
            acat_t = cpool.tile([P, 8], bf16)
            nc.sync.dma_start(out=acat_t[:], in_=Acat[:])
            dlt = cpool.tile([P, max(T_v, 1)], bf16)
            nc.scalar.dma_start(out=dlt[:], in_=dlp[:])
            acc = cpool.tile([P, SLOTS * 132], f32)
            a2sb = cpool.tile([P, SLOTS * 4], bf16)

            with tc.tile_pool(name="tb", bufs=3) as tb, \
                 tc.tile_pool(name="tbp", bufs=2, space="PSUM") as tbp, \
                 tc.tile_pool(name="eg", bufs=3) as eg, \
                 tc.tile_pool(name="ew", bufs=3) as ew, \
                 tc.tile_pool(name="epo", bufs=3, space="PSUM") as epo, \
                 tc.tile_pool(name="ead", bufs=1, space="PSUM") as ead, \
                 tc.tile_pool(name="etp", bufs=2, space="PSUM") as etp:
                # ---- W' cols 128:136 = W @ Acat (contract over out-features)
                wtp = tbp.tile([P, 136], bf16, space="PSUM", tag="ps")
                nc.tensor.transpose(out=wtp[:, 0:128], in_=wprime[:, 0:128],
                                    identity=identb[:])
                wT = tb.tile([P, P], bf16, tag="wT")
                nc.vector.tensor_copy(out=wT[:], in_=wtp[:, 0:128])
                wap = tbp.tile([P, 136], f32, space="PSUM", tag="ps")
                nc.tensor.matmul(out=wap[:, 0:8], lhsT=wT[:], rhs=acat_t[:],
                                 start=True, stop=True)
                nc.vector.tensor_copy(out=wprime[:, 128:136], in_=wap[:, 0:8])

                # ---- a2 table: a_dst for the shard's windows, [d, 4] each
                for b in range(SLOTS // NBLK):
                    xs = tb.tile([P, NBLK * P], bf16, tag="xt")
                    nc.sync.dma_start(
                        out=xs[:], in_=xsT[:, b * NBLK * P:(b + 1) * NBLK * P])
                    a2p = tbp.tile([P, 136], f32, space="PSUM", tag="ps")
                    for j in range(NBLK):
                        nc.tensor.matmul(
                            out=a2p[:, j * 4:(j + 1) * 4],
                            lhsT=xs[:, j * P:(j + 1) * P],
                            rhs=wprime[:, 132:136], start=True, stop=True)
                    g0 = b * NBLK
                    nc.vector.tensor_copy(
                        out=a2sb[:, g0 * 4:(g0 + NBLK) * 4], in_=a2p[:, 0:NBLK * 4])

                # ---- node table build, chunk-major
                for c in range(NCHUNK):
                    for b in range(TPC // NBLK):
                        xt = tb.tile([P, NBLK * P], bf16, tag="xt")
                        off = c * CHUNK + b * NBLK * P
                        nc.sync.dma_start(
                            out=xt[:], in_=xT[:, off:off + NBLK * P])
                        st = tb.tile([P, NBLK * 256], bf16, tag="st")
                        for t in range(NBLK):
                            ps = tbp.tile([P, 136], f32, space="PSUM", tag="ps")
                            nc.tensor.matmul(
                                out=ps[:], lhsT=xt[:, t * P:(t + 1) * P],
                                rhs=wprime[:], start=True, stop=True)
                            if t % 2 == 0:
                                nc.vector.tensor_copy(
                                    out=st[:, t * 256:t * 256 + 132],
                                    in_=ps[:, 0:132])
                            else:
                                nc.scalar.activation(
                                    out=st[:, t * 256:t * 256 + 132],
                                    in_=ps[:, 0:132], func=AF.Copy)
                        stv = st[:].rearrange("p (t c) -> p t c", t=NBLK)
                        nc.vector.tensor_copy(
                            out=stv[:, :, 132:256], in_=stv[:, :, 0:124])
                        nc.sync.dma_start(
                            out=T1c[c][:].rearrange("(p t) c -> p t c", p=P)[
                                :, b * NBLK:(b + 1) * NBLK, :],
                            in_=stv)

                # ---- edge phase, chunk-major, software-pipelined emission
                segs = []
                g1col = 0
                for sg in sched["segs"]:
                    segs.append((sg["c"], sg["Lg"], sg["Lv"], sg["v0"],
                                 g1col, sg["vtile"], sg["runs"]))
                    g1col += sg["Lg"] * 8
                if _PH < 1:
                    segs = []
                state = {}

                def stage_a(i):
                    """gather + Sel one-hot + PE transposes of Sel."""
                    c, Lg, Lv, tcv, gc_, vtile, runs = segs[i]
                    g1it = eg.tile([P, Lg * 8], i16, tag="g1it")
                    nc.scalar.dma_start(
                        out=g1it[:], in_=g1i[:, gc_:gc_ + Lg * 8])
                    g1b = eg.tile([P, Lg * 256], bf16, tag="g1b")
                    g1v = g1b[:].rearrange("p (t c) -> p t c", t=Lg)
                    if _PH < 2:
                        nc.gpsimd.memset(g1b[:], 0)
                    else:
                        nc.gpsimd.dma_gather(
                            g1v, T1c[c][:], g1it[:], Lg * P, Lg * P, 256,
                            single_packet=False)
                    # Sel one-hot per view col, d-major:
                    # sel[e, (d, v)] = (dl[e,v] == d)
                    sel = ew.tile([P, P * Lv], bf16, tag="sel")
                    selv = sel[:].rearrange("p (d t) -> p d t", d=P)
                    nc.vector.tensor_tensor(
                        out=selv, op=AL.is_equal,
                        in0=dlt[:, tcv:tcv + Lv].unsqueeze(1)
                            .to_broadcast([P, P, Lv]),
                        in1=iotB[:].rearrange("p (d t) -> p d t", d=P)[
                            :, :, 0:Lv])
                    tps = []
                    for q0 in range(0, Lv, 8):
                        qn = min(8, Lv - q0)
                        tp = etp.tile([P, 1024], bf16, space="PSUM", tag="tp")
                        for q in range(qn):
                            nc.tensor.transpose(
                                out=tp[:, q * P:(q + 1) * P],
                                in_=selv[:, :, q0 + q], identity=identb[:])
                        tps.append((q0, qn, tp))
                    state[i] = [g1v, selv, tps, None, None]

                def stage_b(i):
                    """evict SelT psums (ACT) + per-edge a_dst matmuls."""
                    c, Lg, Lv, tcv, gc_, vtile, runs = segs[i]
                    g1v, selv, tps, _, _ = state[i]
                    selT = ew.tile([P, Lv * P], bf16, tag="selT", bufs=2)
                    for (q0, qn, tp) in tps:
                        nc.scalar.activation(
                            out=selT[:, q0 * P:(q0 + qn) * P],
                            in_=tp[:, 0:qn * P], func=AF.Copy)
                    adps = ead.tile([P, Lg * 4], f32, space="PSUM", tag="adps")
                    writers = [[] for _ in range(Lg)]
                    for (g, vis) in runs:
                        for vi in vis:
                            writers[vtile[vi]].append((vi, g))
                    for t in range(Lg):
                        for q, (vi, g) in enumerate(writers[t]):
                            nc.tensor.matmul(
                                out=adps[:, t * 4:(t + 1) * 4],
                                lhsT=selT[:, vi * P:(vi + 1) * P],
                                rhs=a2sb[:, g * 4:(g + 1) * 4],
                                start=(q == 0),
                                stop=(q == len(writers[t]) - 1))
                    state[i][2] = None
                    state[i][3] = adps

                def stage_c1(i):
                    """w = exp(max(z, .2z)), z = a_src + a_dst (in adps psum)."""
                    c, Lg, Lv, tcv, gc_, vtile, runs = segs[i]
                    g1v, selv, _, adps, _ = state[i]
                    z = ew.tile([P, Lg * 4], f32, tag="z")
                    nc.vector.tensor_tensor(
                        out=z[:].rearrange("p (t h) -> p t h", t=Lg),
                        in0=g1v[:, :, 128:132],
                        in1=adps[:].rearrange("p (t h) -> p t h", t=Lg),
                        op=AL.add)
                    lr = ew.tile([P, Lg * 4], f32, tag="lr")
                    nc.vector.scalar_tensor_tensor(
                        out=lr[:], in0=z[:], scalar=0.2, in1=z[:],
                        op0=AL.mult, op1=AL.max)
                    nc.scalar.activation(
                        out=g1v[:, :, 128:132],
                        in_=lr[:].rearrange("p (t h) -> p t h", t=Lg),
                        func=AF.Exp)

                def stage_c2(i):
                    """messages in place, aggregate per run, accumulate."""
                    c, Lg, Lv, tcv, gc_, vtile, runs = segs[i]
                    g1v, selv, _, adps, _ = state[i]
                    nc.vector.tensor_tensor(
                        out=g1v[:, :, 0:128].rearrange(
                            "p t (cc h) -> p t cc h", h=4),
                        op=AL.mult,
                        in0=g1v[:, :, 0:128].rearrange(
                            "p t (cc h) -> p t cc h", h=4),
                        in1=g1v[:, :, 128:132].unsqueeze(2)
                            .to_broadcast([P, Lg, 32, 4]))
                    pos = []
                    for (g, vis) in runs:
                        po = epo.tile([P, 132], f32, space="PSUM", tag="po")
                        for q, vi in enumerate(vis):
                            nc.tensor.matmul(
                                out=po[:], lhsT=selv[:, :, vi],
                                rhs=g1v[:, vtile[vi], 0:132],
                                start=(q == 0), stop=(q == len(vis) - 1))
                        pos.append((g, po))
                    state[i][4] = pos

                def stage_d(i):
                    """accumulate psums into SBUF acc."""
                    c = segs[i][0]
                    for (g, po) in state[i][4]:
                        aslice = acc[:, g * 132:(g + 1) * 132]
                        if c == first_c[g]:
                            nc.vector.tensor_copy(out=aslice, in_=po[:])
                        else:
                            nc.vector.tensor_tensor(
                                out=aslice, in0=aslice, in1=po[:], op=AL.add)
                    del state[i]

                def emit_out(runs):
                    """normalize + bias + store finished slots."""
                    gs = [g for (g, _) in runs]
                    g0, gn = gs[0], len(gs)
                    assert gs == list(range(g0, g0 + gn))
                    accw = acc[:, g0 * 132:(g0 + gn) * 132].rearrange(
                        "p (j m) -> p j m", j=gn)
                    rec = ew.tile([P, gn * 4], f32, tag="rec")
                    nc.vector.reciprocal(
                        out=rec[:].rearrange("p (j h) -> p j h", j=gn),
                        in_=accw[:, :, 128:132])
                    ot = ew.tile([P, gn * P], f32, tag="ot")
                    otv = ot[:].rearrange("p (j c) -> p j c", j=gn)
                    nc.vector.tensor_tensor(
                        out=otv.rearrange("p j (cc h) -> p j cc h", h=4),
                        in0=accw[:, :, 0:128].rearrange(
                            "p j (cc h) -> p j cc h", h=4),
                        in1=rec[:].rearrange("p (j h) -> p j h", j=gn)
                            .unsqueeze(2).to_broadcast([P, gn, CDIM, 4]),
                        op=AL.mult)
                    nc.vector.tensor_tensor(
                        out=otv, in0=otv,
                        in1=bias_t[:].unsqueeze(1).to_broadcast([P, gn, P]),
                        op=AL.add)
                    nc.sync.dma_start(
                        out=outp[g0 * P:(g0 + gn) * P, :].rearrange(
                            "(j p) c -> p j c", p=P),
                        in_=otv)

                ns = len(segs)
                if ns:
                    stage_a(0)
                    stage_b(0)
                    if ns > 1:
                        stage_a(1)
                for i in range(ns):
                    stage_c1(i)
                    stage_c2(i)
                    if i + 1 < ns:
                        stage_b(i + 1)
                    if i + 2 < ns:
                        stage_a(i + 2)
                    stage_d(i)
                    if segs[i][0] == NCHUNK - 1:
                        emit_out(segs[i][6])
                if ns:
                    for g in range(SLOTS):
                        if mx[g, NCHUNK - 1] == 0 and mx[g].sum() > 0:
                            emit_out([(g, [])])

    nc.compile()
    if not os.environ.get("BASS_NO_WAITSPLIT"):
        _split_multi_waits(nc)
    return nc


# ---------------------------------------------------------------------------
def _prep_and_build(x, edge_index, W, att_src, att_dst, bias):
    cores, sched = _host_prep(np.asarray(x), np.asarray(edge_index))
    nc = _build_nc(sched)

    x = np.asarray(x, np.float32)
    xpad = np.zeros((NP_, IN_DIM), BF16)
    xpad[:N] = x.astype(BF16)
    xT = np.ascontiguousarray(xpad.T)

    # out-features reordered c-major: new col c*4+h = old col h*32+c
    perm = np.array([h * CDIM + c for c in range(CDIM) for h in range(HEADS)])
    a_s = np.asarray(att_src, np.float32)
    a_d = np.asarray(att_dst, np.float32)
    Acat = np.zeros((P, 8), BF16)
    for h in range(HEADS):
        for c in range(CDIM):
            Acat[c * HEADS + h, h] = a_s[h, c]
            Acat[c * HEADS + h, 4 + h] = a_d[h, c]
    biasr = np.tile(np.asarray(bias, np.float32)[perm][None, :], (P, 1))
    LCMAX = max(sg["Lv"] for sg in sched["segs"])
    iotB = np.tile(
        np.repeat(np.arange(P), LCMAX).astype(BF16)[None, :], (P, 1))
    Wf = np.ascontiguousarray(np.asarray(W, np.float32)[:, perm].astype(BF16))

    in_maps = []
    for k in range(NCORES):
        g1w, dlt = _pack_core_arrays(cores[k], sched)
        nodes = (sched["grp"][:, k][:, None] * P + np.arange(P)[None, :]).reshape(-1)
        xsT = np.ascontiguousarray(xpad[nodes].T)
        in_maps.append({
            "xT": xT, "xsT": xsT, "W": Wf, "Acat": Acat, "biasr": biasr,
            "iotB": iotB,
            "g1i": np.ascontiguousarray(g1w),
            "dlp": np.ascontiguousarray(dlt),
        })
    return nc, in_maps, sched


def _assemble(results, sched):
    perm = np.array([h * CDIM + c for c in range(CDIM) for h in range(HEADS)])
    inv = np.empty(P, np.int64)
    inv[perm] = np.arange(P)
    full = np.zeros((NP_, P), np.float32)
    grp = sched["grp"]
    for k in range(NCORES):
        o = np.asarray(results[k]["out"])[:, inv]   # [SHARD, 128], (h,c) order
        wins = grp[:, k]                            # window id per slot
        full[(wins[:, None] * P + np.arange(P)[None, :]).reshape(-1)] = o
    return full[:N]


def kernel(**inputs):
    x = inputs["x"]
    edge_index = inputs["edge_index"]
    nc, in_maps, sched = _prep_and_build(
        x, edge_index, inputs["W"], inputs["att_src"], inputs["att_dst"],
        inputs["bias"])
    res = run_bass_kernel_spmd(nc, in_maps, core_ids=list(range(NCORES)))
    return _assemble(res.results, sched)
